# revision 12
# baseline (speedup 1.0000x reference)
"""BERT self-attention kernel for Trainium2, 8-core SPMD.

Problem: hidden_states [S=2048, B=2, H=1024], 16 heads x 64, fp32.
Sharding: core i handles batch b = i//4 and head-group hg = i%4
(4 heads = 256 contiguous columns of Wq/Wk/Wv). Each core:

  hsT   = hs.T     (pre-transposed + bf16-cast on HOST; contiguous
                    [si][p, ko, s] blocks so each si is one 1MB DMA)
  qT/kT = W.T @ hsT (+bias fused into the PSUM->SBUF copy)   [d, s] bf16
  v     = hsT.T @ Wv (+bias via K=1 matmul)   [t, d] fp8, + ones col
  scT   = kT_h.T @ qT_h                 [t, s] K=64; both heads of a
                                        pair issued back-to-back at PE
                                        row offsets 0/64 (tile_position)
  expT  = exp(scT / 8) -> fp8e4m3       (ScalarE activation, or DVE
                                        Schraudolph int8-bitcast)
  ctxT_aug = v_aug.T @ expT             fp8 DoubleRow matmul: 2 t-chunks
                                        (256 keys) per instruction at
                                        ~2x bf16 throughput; [65, s] f32
                                        psum; row 64 = sumexp
  out   = transpose(ctxT_aug)[:, 0:64] * (1 / col 64)

Softmax normalization is deferred past the PV matmul (softmax is
shift-invariant and scores are O(1) here, so no max-subtraction).
PE (TensorE) is the critical engine (~137us busy: QK proj 27.6,
V proj 14.1, scores 28.4, PV 31, transposes ~6, rest LDW/sync);
ScalarE (10/16 exp chunks) and DVE (6-7/16 + copies) sit at ~90us.
Startup: input DMAs (5.5MB at ~380GB/s) ordered so the K0/si0
projection inputs land first (wk half 0, hsT si0, wq half 0, rest).
"""

import numpy as np

S = 2048
B = 2
H = 1024
NH = 16
HD = 64
P = 128
HG = 256          # head-group width (4 heads) per core
NHEADS_CORE = 4
SBLK = 512        # query block
NB = S // SBLK    # 4
NTCH = S // P     # 16 key chunks
KO = H // P       # 8 contraction chunks for projections
N_CORES = 8

PV_FP8 = True     # fp8 DoubleRow PV matmul (2x PE throughput on PV)
TRANSPOSELESS = False  # normalize via K=1 broadcast matmul, output [d, s]
VW = 80           # per-head v row width (64 dims + ones col + pad, %16==0)

# Schraudolph exp on VectorE: bitcast(int(x*CM + CB)) approximates
# exp(x/8); CB calibrated for zero mean log-ratio vs exp.
# bf16/int16 variant (PV_FP8=False):
CM16 = 23.083120654223414
CB16 = 16248.75
# fp8e4m3/int8 variant (PV_FP8=True):
CM8 = 1.4426950408889634
CB8 = 55.54
DVE_SET = (1, 3, 5, 7, 11, 15)   # t-chunks whose exp runs on VectorE

_CACHE = {}


def _build_nc(with_bias=True):
    import concourse.mybir as mybir
    import concourse.tile as tile
    from concourse import bacc

    f32 = mybir.dt.float32
    f16 = mybir.dt.float16
    bf16 = mybir.dt.bfloat16
    f8 = mybir.dt.float8e4
    i16 = mybir.dt.int16
    i8 = mybir.dt.int8
    Exp = mybir.ActivationFunctionType.Exp
    Mult = mybir.AluOpType.mult
    Add = mybir.AluOpType.add
    DR = mybir.MatmulPerfMode.DoubleRow

    e_dt = f8 if PV_FP8 else bf16

    nc = bacc.Bacc(None, target_bir_lowering=False)

    # hsT: host pre-transposed, [si][p, ko, s] contiguous per si block
    hsT_d = nc.dram_tensor("hsT", [NB, P, KO, SBLK], bf16, kind="ExternalInput")
    wq_d = nc.dram_tensor("wq", [P, KO, HG], bf16, kind="ExternalInput")
    wk_d = nc.dram_tensor("wk", [P, KO, HG], bf16, kind="ExternalInput")
    wv_d = nc.dram_tensor("wv", [P, KO, HG], bf16, kind="ExternalInput")
    bq_d = nc.dram_tensor("bq", [HG], f32, kind="ExternalInput")
    bk_d = nc.dram_tensor("bk", [HG], f32, kind="ExternalInput")
    bv_d = nc.dram_tensor("bv", [HG], bf16, kind="ExternalInput")
    ones_d = nc.dram_tensor("ones", [NTCH * NHEADS_CORE * P], e_dt, kind="ExternalInput")
    onesb_d = nc.dram_tensor("onesb", [P], bf16, kind="ExternalInput")
    idf_d = nc.dram_tensor("idf", [P, P], f32, kind="ExternalInput")
    if TRANSPOSELESS:
        out_d = nc.dram_tensor("out", [NHEADS_CORE, HD, S], f32,
                               kind="ExternalOutput")
    else:
        out_d = nc.dram_tensor("out", [S, HG], f32, kind="ExternalOutput")

    with tile.TileContext(nc) as tc:
        with (
            tc.tile_pool(name="const", bufs=1) as cst,
            tc.tile_pool(name="qkv", bufs=1) as qkv,
        ):
            bcol_q = cst.tile([P, 2], f32)
            bcol_k = cst.tile([P, 2], f32)
            bv_row = cst.tile([1, HG], bf16)
            ones_row = cst.tile([1, P], bf16)

            # k0/k1 full [d, s]; q split per s-block; v one tile
            qkT = {}
            for nm in ("k0", "k1"):
                qkT[nm] = qkv.tile([P, S], bf16, tag=f"T{nm}", name=f"T{nm}")
            qT_s = {}
            for pair in range(2):
                for si in range(NB):
                    qT_s[(pair, si)] = qkv.tile(
                        [P, SBLK], bf16, tag=f"qT{pair}{si}", name=f"qT{pair}{si}"
                    )
            # v: [p, t-chunk, head, VW]; col 64 = ones (sumexp trick);
            # VW=80 keeps the t-chunk stride %16==0 for DoubleRow APs.
            v_t = qkv.tile([P, NTCH, NHEADS_CORE, VW], e_dt, tag="v", name="v")

            hsT_q = [qkv.tile([P, KO, SBLK], bf16, tag=f"hsT{si}",
                              name=f"hsT{si}") for si in range(NB)]

            # attention pools
            ep = tc.alloc_tile_pool(name="expt", bufs=2)
            op = tc.alloc_tile_pool(name="outs", bufs=3)
            scp = tc.alloc_tile_pool(name="sc_ps", bufs=3, space="PSUM")
            cxp = tc.alloc_tile_pool(name="cx_ps", bufs=2, space="PSUM")

            # ---- input DMAs, ordered for fast pipeline start ---------
            # All on the sync HWDGE ring (keeps ScalarE's queue free for
            # activations); ordered so K0/si0 can begin ASAP.
            w_sb = {}
            for nm in ("k", "q"):
                w_sb[nm] = [cst.tile([P, KO, P], bf16, tag=f"w{nm}{m}",
                                     name=f"w{nm}{m}") for m in range(2)]
            w_sb["v"] = cst.tile([P, KO, HG], bf16, tag="wv", name="wv")
            # K0/si0 critical path first: wk half 0, then hsT si0 in two
            # chunks, then wq half 0; the rest streams behind.
            nc.sync.dma_start(w_sb["k"][0][:], wk_d[:, :, 0:P])
            nc.sync.dma_start(hsT_q[0][:, 0:KO // 2, :], hsT_d[0, :, 0:KO // 2, :])
            nc.sync.dma_start(hsT_q[0][:, KO // 2:, :], hsT_d[0, :, KO // 2:, :])
            nc.sync.dma_start(w_sb["q"][0][:], wq_d[:, :, 0:P])
            if with_bias:
                nc.sync.dma_start(bcol_k[:], bk_d.rearrange("(m p) -> p m", p=P))
                nc.sync.dma_start(bcol_q[:], bq_d.rearrange("(m p) -> p m", p=P))
            for si in range(1, NB):
                nc.sync.dma_start(hsT_q[si][:], hsT_d[si])
            nc.sync.dma_start(w_sb["v"][:], wv_d[:])
            nc.sync.dma_start(w_sb["k"][1][:], wk_d[:, :, P:HG])
            nc.sync.dma_start(w_sb["q"][1][:], wq_d[:, :, P:HG])
            if with_bias:
                nc.sync.dma_start(bv_row[:], bv_d[None, :])
            nc.sync.dma_start(ones_row[:], onesb_d[None, :])
            # ones column of v (sumexp accumulator)
            nc.sync.dma_start(
                v_t[:, :, :, HD:HD + 1],
                ones_d.rearrange("(p t h) -> p t h", p=P, t=NTCH)[:, :, :, None],
            )
            if not TRANSPOSELESS:
                ident = cst.tile([P, P], f32)
                nc.sync.dma_start(ident[:], idf_d[:])
            out_v = None
            if not TRANSPOSELESS:
                out_v = out_d.rearrange("(nb c p) hh -> p nb c hh", p=P, c=NB)

            Ident = mybir.ActivationFunctionType.Identity

            def qk_proj(w, bcol, m, si, dst, use_act=False):
                pst = scp.tile([P, 2, SBLK], f32, tag="sc",
                               name="qk_ps")[:, 0, :]
                for ko in range(KO):
                    nc.tensor.matmul(
                        pst,
                        w[m][:, ko, :],
                        hsT_q[si][:, ko, :],
                        start=(ko == 0), stop=(ko == KO - 1),
                    )
                if not with_bias:
                    if use_act:
                        nc.scalar.copy(dst, pst)
                    else:
                        nc.vector.tensor_copy(dst, pst)
                elif use_act:
                    nc.scalar.activation(dst, pst, Ident,
                                         bias=bcol[:, m:m + 1])
                else:
                    nc.vector.tensor_scalar_add(dst, pst, bcol[:, m:m + 1])

            def v_proj(to):
                pst = scp.tile([P, 2, SBLK], f32, tag="sc",
                               name="v_ps")[:, 0, 0:HG]
                for ko in range(KO):
                    nc.tensor.matmul(
                        pst,
                        hsT_q[to // 4][:, ko, (to % 4) * P:(to % 4 + 1) * P],
                        w_sb["v"][:, ko, :],
                        start=(ko == 0),
                        stop=(not with_bias and ko == KO - 1),
                    )
                if with_bias:
                    nc.tensor.matmul(
                        pst, ones_row[0:1, :], bv_row[:],
                        start=False, stop=True,
                    )
                nc.vector.tensor_copy(
                    v_t[:, to, :, 0:HD],
                    pst.rearrange("p (h d) -> p h d", d=HD),
                )

            # ---- attention pipeline ----------------------------------
            def _attention_pair(pair, unit_hook=None):
                kTt = qkT[f"k{pair}"]
                dve_set = DVE_SET
                for sb_i in range(NB):
                    qTt = qT_s[(pair, sb_i)]
                    expt = ep.tile([P, NTCH, 2, SBLK], e_dt, tag="expt",
                                   name="expt")
                    ctxps = [cxp.tile([HD + 1, SBLK], f32, tag="cx",
                                      name=f"ctx{h2}") for h2 in range(2)]

                    def scores_exp(t):
                        sc = scp.tile([P, 2, SBLK], f32, tag="sc",
                                      name="sc")
                        for h2 in range(2):
                            po = 64 * h2
                            nc.tensor.matmul(
                                sc[:, h2, :],
                                kTt[po:po + HD, t * P:(t + 1) * P],
                                qTt[po:po + HD, :],
                                start=True, stop=True,
                                tile_position=(po, 0),
                            )
                        if t in dve_set:
                            if PV_FP8:
                                nc.vector.tensor_scalar(
                                    expt[:, t, :, :].bitcast(i8), sc[:],
                                    CM8, CB8, Mult, Add,
                                )
                            else:
                                nc.vector.tensor_scalar(
                                    expt[:, t, :, :].bitcast(i16), sc[:],
                                    CM16, CB16, Mult, Add,
                                )
                        else:
                            nc.scalar.activation(
                                expt[:, t, :, :], sc[:], Exp, scale=0.125,
                            )

                    def ctx_batch(ts):
                        # ts: even-aligned range; fp8 path consumes t-chunk
                        # PAIRS via DoubleRow (256-key contraction per MM)
                        for h2 in range(2):
                            head = pair * 2 + h2
                            if PV_FP8:
                                for tp in range(ts.start, ts.stop, 2):
                                    nc.tensor.matmul(
                                        ctxps[h2][:],
                                        v_t[:, tp:tp + 2, head, 0:HD + 1],
                                        expt[:, tp:tp + 2, h2, :],
                                        start=(tp == 0),
                                        stop=(tp == NTCH - 2),
                                        perf_mode=DR,
                                        skip_group_check=True,
                                    )
                            else:
                                for t in ts:
                                    nc.tensor.matmul(
                                        ctxps[h2][:],
                                        v_t[:, t, head, 0:HD + 1],
                                        expt[:, t, h2, :],
                                        start=(t == 0), stop=(t == NTCH - 1),
                                        skip_group_check=True,
                                    )

                    for t in range(NTCH):
                        scores_exp(t)
                        if unit_hook is not None:
                            unit_hook(sb_i, t)
                        if t in (5, 9, 13):
                            ctx_batch(range(t - 5, t - 1))
                        elif t == 15:
                            ctx_batch(range(12, 14))
                    ctx_batch(range(14, NTCH))

                    if TRANSPOSELESS:
                        # normalize in [d, s] layout: rec_row = 1/sumexp
                        # (fp16), broadcast across partitions via a K=1
                        # matmul, multiply, DMA out transposed.
                        for h2 in range(2):
                            head = pair * 2 + h2
                            ctxT = op.tile([HD + 1, SBLK], f32, tag="ctxT",
                                           name="ctxT")
                            nc.vector.tensor_copy(ctxT[:], ctxps[h2][:])
                            rec_row = op.tile([1, SBLK], f16, tag="rec",
                                              name="rec")
                            with nc.allow_low_precision(
                                    reason="fp16 reciprocal row; 2^-11 "
                                    "relative error is within budget"):
                                nc.vector.reciprocal(
                                    rec_row[:], ctxps[h2][HD:HD + 1, :])
                            recb = scp.tile([P, 2, SBLK], f32, tag="sc",
                                            name="recb")[0:HD, 0, :]
                            nc.tensor.matmul(
                                recb, ones_row[0:1, 0:HD], rec_row[:],
                                start=True, stop=True,
                            )
                            osbT = op.tile([HD, SBLK], f32, tag="osbT",
                                           name="osbT")
                            nc.vector.tensor_tensor(
                                osbT[:], ctxT[0:HD, :], recb,
                                mybir.AluOpType.mult,
                            )
                            nc.sync.dma_start(
                                out_d[head, :, sb_i * SBLK:(sb_i + 1) * SBLK],
                                osbT[:],
                            )
                    else:
                        ctxTs = []
                        for h2 in range(2):
                            ctxT = op.tile([HD + 1, SBLK], f32, tag="ctxT",
                                           name="ctxT")
                            nc.vector.tensor_copy(ctxT[:], ctxps[h2][:])
                            ctxTs.append(ctxT)
                        for h2 in range(2):
                            head = pair * 2 + h2
                            ctxT = ctxTs[h2]
                            ot = cxp.tile([P, NB, HD + 1], f32, tag="cx",
                                          name="ot")
                            for c in range(NB):
                                nc.tensor.transpose(
                                    ot[:, c, :],
                                    ctxT[:, c * P:(c + 1) * P],
                                    ident[0:HD + 1, 0:HD + 1],
                                )
                            rec = op.tile([P, NB, 1], f32, tag="rec",
                                          name="rec")
                            nc.vector.reciprocal(rec[:], ot[:, :, HD:HD + 1])
                            osb = op.tile([P, NB, HD], f32, tag="osb",
                                          name="osb")
                            nc.vector.tensor_tensor(
                                osb[:], ot[:, :, 0:HD],
                                rec.to_broadcast([P, NB, HD]),
                                mybir.AluOpType.mult,
                            )
                            nc.sync.dma_start(
                                out_v[:, sb_i, :, head * HD:(head + 1) * HD],
                                osb[:],
                            )

            # ---- emission --------------------------------------------
            for si in range(NB):
                qk_proj(w_sb["k"], bcol_k, 0, si,
                        qkT["k0"][:, si * SBLK:(si + 1) * SBLK],
                        use_act=True)
            qk_proj(w_sb["q"], bcol_q, 0, 0, qT_s[(0, 0)][:],
                    use_act=True)

            def _hook_p0(sb_i, t):
                if sb_i == 0 and 1 <= t <= 8:
                    # two V chains per unit; group g is complete
                    # before ctx needs it two units later
                    v_proj(2 * (t - 1))
                    v_proj(2 * (t - 1) + 1)
                elif sb_i == 0 and 9 <= t <= 11:
                    qk_proj(w_sb["q"], bcol_q, 0, t - 8, qT_s[(0, t - 8)][:])
                elif sb_i == 1 and 1 <= t <= 4:
                    si = t - 1
                    qk_proj(w_sb["k"], bcol_k, 1, si,
                            qkT["k1"][:, si * SBLK:(si + 1) * SBLK])
                elif sb_i == 2 and 1 <= t <= 4:
                    qk_proj(w_sb["q"], bcol_q, 1, t - 1, qT_s[(1, t - 1)][:])

            _attention_pair(0, _hook_p0)
            _attention_pair(1)

            for _pool in (cxp, scp, op, ep):
                _pool.release()
    nc.compile()
    return nc


def _get_nc(with_bias=True):
    key = f"nc_{with_bias}"
    if key not in _CACHE:
        _CACHE[key] = _build_nc(with_bias=with_bias)
    return _CACHE[key]


def _kernel_np(hidden_states, attention_mask, Wq, bq, Wk, bk, Wv, bv):
    """Numpy fallback for the general (non-zero attention_mask) case."""
    S_, B_, H_ = hidden_states.shape
    hd = H_ // NH

    def split(x):
        return x.reshape(S_, B_ * NH, hd).transpose(1, 0, 2)

    q = split(hidden_states @ Wq + bq)
    k = split(hidden_states @ Wk + bk)
    v = split(hidden_states @ Wv + bv)
    scores = np.einsum("nsd,ntd->nst", q, k).reshape(B_, NH, S_, S_)
    scores = scores / np.sqrt(np.float32(hd)) + attention_mask
    scores = scores - scores.max(axis=-1, keepdims=True)
    e = np.exp(scores)
    probs = (e / e.sum(axis=-1, keepdims=True)).reshape(B_ * NH, S_, S_)
    ctx = np.einsum("nst,ntd->nsd", probs.astype(np.float32), v)
    return ctx.transpose(1, 0, 2).reshape(S_, B_, H_).astype(np.float32)


def kernel(hidden_states, attention_mask, Wq, bq, Wk, bk, Wv, bv, _trace=False, _tmpdir=None):
    import ml_dtypes
    bf = ml_dtypes.bfloat16
    f16 = np.float16
    f8 = ml_dtypes.float8_e4m3fn
    e_np = f8 if PV_FP8 else bf
    hidden_states = np.ascontiguousarray(hidden_states, dtype=np.float32)
    if attention_mask is not None and np.any(attention_mask):
        return _kernel_np(hidden_states, attention_mask, Wq, bq, Wk, bk, Wv, bv)

    from concourse.bass_utils import run_bass_kernel_spmd

    with_bias = bool(np.any(bq) or np.any(bk) or np.any(bv))
    nc = _get_nc(with_bias=with_bias)
    ones = np.ones(NTCH * NHEADS_CORE * P, e_np)
    onesb = np.ones(P, bf)
    idf = np.eye(P, dtype=np.float32)
    hs_16 = hidden_states.astype(bf)
    wq_16 = np.asarray(Wq, np.float32).astype(bf)
    wk_16 = np.asarray(Wk, np.float32).astype(bf)
    wv_16 = np.asarray(Wv, np.float32).astype(bf)

    def warr(w, c0):
        # [H, 256] -> [p, ko, m] contiguous
        return np.ascontiguousarray(
            w[:, c0:c0 + HG].reshape(KO, P, HG).transpose(1, 0, 2))

    in_maps = []
    for core in range(N_CORES):
        b = core // 4
        hg = core % 4
        c0 = hg * HG
        # hsT: [si, p, ko, s] with hs[si*512+s, ko*128+p]
        hsT = np.ascontiguousarray(
            hs_16[:, b, :].reshape(NB, SBLK, KO, P).transpose(0, 3, 2, 1))
        in_maps.append({
            "hsT": hsT,
            "wq": warr(wq_16, c0),
            "wk": warr(wk_16, c0),
            "wv": warr(wv_16, c0),
            "bq": np.ascontiguousarray(bq[c0:c0 + HG], dtype=np.float32),
            "bk": np.ascontiguousarray(bk[c0:c0 + HG], dtype=np.float32),
            "bv": np.ascontiguousarray(np.asarray(bv[c0:c0 + HG], np.float32).astype(bf)),
            "ones": ones,
            "onesb": onesb,
            "idf": idf,
        })
    res = None
    last_err = None
    for _attempt in range(3):
        try:
            res = run_bass_kernel_spmd(
                nc, in_maps, core_ids=list(range(N_CORES)), trace=_trace,
                tmpdir=_tmpdir,
            )
            break
        except Exception as e:  # transient NRT/device hiccups: retry
            last_err = e
            import time as _time
            _time.sleep(2.0)
    if res is None:
        raise last_err
    out = np.empty((S, B, H), np.float32)
    for core in range(N_CORES):
        b = core // 4
        hg = core % 4
        r = res.results[core]["out"]
        if TRANSPOSELESS:
            # r: [4, 64, S] -> [S, 256]
            out[:, b, hg * HG:(hg + 1) * HG] = r.reshape(HG, S).T
        else:
            out[:, b, hg * HG:(hg + 1) * HG] = r
    if _trace:
        _CACHE["last_results"] = res
    return out


# revision 13
# speedup vs baseline: 1.0344x; 1.0344x over previous
"""BERT self-attention kernel for Trainium2, 8-core SPMD.

Problem: hidden_states [S=2048, B=2, H=1024], 16 heads x 64, fp32.
Sharding: core i handles batch b = i//4 and head-group hg = i%4
(4 heads = 256 contiguous columns of Wq/Wk/Wv). Each core:

  hsT   = hs.T     (pre-transposed + bf16-cast on HOST; contiguous
                    [si][p, ko, s] blocks so each si is one 1MB DMA)
  qT/kT = W.T @ hsT (+bias fused into the PSUM->SBUF copy)   [d, s] bf16
  v     = hsT.T @ Wv (+bias via K=1 matmul)   [t, d] fp8, + ones col
  scT   = kT_h.T @ qT_h                 [t, s] K=64; both heads of a
                                        pair issued back-to-back at PE
                                        row offsets 0/64 (tile_position)
  expT  = exp(scT / 8) -> fp8e4m3       (ScalarE activation, or DVE
                                        Schraudolph int8-bitcast)
  ctxT_aug = v_aug.T @ expT             fp8 DoubleRow matmul: 2 t-chunks
                                        (256 keys) per instruction at
                                        ~2x bf16 throughput; [65, s] f32
                                        psum; row 64 = sumexp
  out   = transpose(ctxT_aug)[:, 0:64] * (1 / col 64)

Softmax normalization is deferred past the PV matmul (softmax is
shift-invariant and scores are O(1) here, so no max-subtraction).
PE (TensorE) is the critical engine (~137us busy: QK proj 27.6,
V proj 14.1, scores 28.4, PV 31, transposes ~6, rest LDW/sync);
ScalarE (10/16 exp chunks) and DVE (6-7/16 + copies) sit at ~90us.
Startup: input DMAs (5.5MB at ~380GB/s) ordered so the K0/si0
projection inputs land first (wk half 0, hsT si0, wq half 0, rest).
"""

import numpy as np

S = 2048
B = 2
H = 1024
NH = 16
HD = 64
P = 128
HG = 256          # head-group width (4 heads) per core
NHEADS_CORE = 4
SBLK = 512        # query block
NB = S // SBLK    # 4
NTCH = S // P     # 16 key chunks
KO = H // P       # 8 contraction chunks for projections
N_CORES = 8

PV_FP8 = True     # fp8 DoubleRow PV matmul (2x PE throughput on PV)
TRANSPOSELESS = False  # normalize via K=1 broadcast matmul, output [d, s]
VW = 80           # per-head v row width (64 dims + ones col + pad, %16==0)

# Schraudolph exp on VectorE: bitcast(int(x*CM + CB)) approximates
# exp(x/8); CB calibrated for zero mean log-ratio vs exp.
# bf16/int16 variant (PV_FP8=False):
CM16 = 23.083120654223414
CB16 = 16248.75
# fp8e4m3/int8 variant (PV_FP8=True):
CM8 = 1.4426950408889634
CB8 = 55.54
DVE_SET = (1, 3, 5, 7, 11, 15)     # pair-0 t-chunks on VectorE
DVE_SET_P1 = (1, 3, 5, 7, 9, 11, 15)  # pair-1 (no hook work; ACT paces)

_CACHE = {}


def _build_nc(with_bias=True):
    import concourse.mybir as mybir
    import concourse.tile as tile
    from concourse import bacc

    f32 = mybir.dt.float32
    f16 = mybir.dt.float16
    bf16 = mybir.dt.bfloat16
    f8 = mybir.dt.float8e4
    i16 = mybir.dt.int16
    i8 = mybir.dt.int8
    Exp = mybir.ActivationFunctionType.Exp
    Mult = mybir.AluOpType.mult
    Add = mybir.AluOpType.add
    DR = mybir.MatmulPerfMode.DoubleRow

    e_dt = f8 if PV_FP8 else bf16

    nc = bacc.Bacc(None, target_bir_lowering=False)

    # hsT: host pre-transposed, [si][p, ko, s] contiguous per si block
    hsT_d = nc.dram_tensor("hsT", [NB, P, KO, SBLK], bf16, kind="ExternalInput")
    wq_d = nc.dram_tensor("wq", [P, KO, HG], bf16, kind="ExternalInput")
    wk_d = nc.dram_tensor("wk", [P, KO, HG], bf16, kind="ExternalInput")
    wv_d = nc.dram_tensor("wv", [P, KO, HG], bf16, kind="ExternalInput")
    bq_d = nc.dram_tensor("bq", [HG], f32, kind="ExternalInput")
    bk_d = nc.dram_tensor("bk", [HG], f32, kind="ExternalInput")
    bv_d = nc.dram_tensor("bv", [HG], bf16, kind="ExternalInput")
    ones_d = nc.dram_tensor("ones", [NTCH * NHEADS_CORE * P], e_dt, kind="ExternalInput")
    onesb_d = nc.dram_tensor("onesb", [P], bf16, kind="ExternalInput")
    idf_d = nc.dram_tensor("idf", [P, P], f32, kind="ExternalInput")
    if TRANSPOSELESS:
        out_d = nc.dram_tensor("out", [NHEADS_CORE, HD, S], f32,
                               kind="ExternalOutput")
    else:
        out_d = nc.dram_tensor("out", [S, HG], f32, kind="ExternalOutput")

    with tile.TileContext(nc) as tc:
        with (
            tc.tile_pool(name="const", bufs=1) as cst,
            tc.tile_pool(name="qkv", bufs=1) as qkv,
        ):
            bcol_q = cst.tile([P, 2], f32)
            bcol_k = cst.tile([P, 2], f32)
            bv_row = cst.tile([1, HG], bf16)
            ones_row = cst.tile([1, P], bf16)

            # k0/k1 full [d, s]; q split per s-block; v one tile
            qkT = {}
            for nm in ("k0", "k1"):
                qkT[nm] = qkv.tile([P, S], bf16, tag=f"T{nm}", name=f"T{nm}")
            qT_s = {}
            for pair in range(2):
                for si in range(NB):
                    qT_s[(pair, si)] = qkv.tile(
                        [P, SBLK], bf16, tag=f"qT{pair}{si}", name=f"qT{pair}{si}"
                    )
            # v: [p, t-chunk, head, VW]; col 64 = ones (sumexp trick);
            # VW=80 keeps the t-chunk stride %16==0 for DoubleRow APs.
            v_t = qkv.tile([P, NTCH, NHEADS_CORE, VW], e_dt, tag="v", name="v")

            hsT_q = [qkv.tile([P, KO, SBLK], bf16, tag=f"hsT{si}",
                              name=f"hsT{si}") for si in range(NB)]

            # attention pools
            ep = tc.alloc_tile_pool(name="expt", bufs=2)
            op = tc.alloc_tile_pool(name="outs", bufs=3)
            scp = tc.alloc_tile_pool(name="sc_ps", bufs=3, space="PSUM")
            cxp = tc.alloc_tile_pool(name="cx_ps", bufs=2, space="PSUM")

            # ---- input DMAs, ordered for fast pipeline start ---------
            # All on the sync HWDGE ring (keeps ScalarE's queue free for
            # activations); ordered so K0/si0 can begin ASAP.
            w_sb = {}
            for nm in ("k", "q"):
                w_sb[nm] = [cst.tile([P, KO, P], bf16, tag=f"w{nm}{m}",
                                     name=f"w{nm}{m}") for m in range(2)]
            w_sb["v"] = cst.tile([P, KO, HG], bf16, tag="wv", name="wv")
            # K0/si0 critical path first: wk half 0, then hsT si0 in two
            # chunks, then wq half 0; the rest streams behind.
            nc.sync.dma_start(w_sb["k"][0][:], wk_d[:, :, 0:P])
            nc.scalar.dma_start(hsT_q[0][:, KO // 2:, :], hsT_d[0, :, KO // 2:, :])
            nc.sync.dma_start(hsT_q[0][:, 0:KO // 2, :], hsT_d[0, :, 0:KO // 2, :])
            nc.scalar.dma_start(w_sb["q"][0][:], wq_d[:, :, 0:P])
            if with_bias:
                nc.sync.dma_start(bcol_k[:], bk_d.rearrange("(m p) -> p m", p=P))
                nc.sync.dma_start(bcol_q[:], bq_d.rearrange("(m p) -> p m", p=P))
            for si in range(1, NB):
                nc.sync.dma_start(hsT_q[si][:], hsT_d[si])
            nc.sync.dma_start(w_sb["v"][:], wv_d[:])
            nc.sync.dma_start(w_sb["k"][1][:], wk_d[:, :, P:HG])
            nc.sync.dma_start(w_sb["q"][1][:], wq_d[:, :, P:HG])
            if with_bias:
                nc.sync.dma_start(bv_row[:], bv_d[None, :])
            nc.sync.dma_start(ones_row[:], onesb_d[None, :])
            # ones column of v (sumexp accumulator)
            nc.sync.dma_start(
                v_t[:, :, :, HD:HD + 1],
                ones_d.rearrange("(p t h) -> p t h", p=P, t=NTCH)[:, :, :, None],
            )
            if not TRANSPOSELESS:
                ident = cst.tile([P, P], f32)
                nc.sync.dma_start(ident[:], idf_d[:])
            out_v = None
            if not TRANSPOSELESS:
                out_v = out_d.rearrange("(nb c p) hh -> p nb c hh", p=P, c=NB)

            Ident = mybir.ActivationFunctionType.Identity

            def qk_proj(w, bcol, m, si, dst, use_act=False):
                pst = scp.tile([P, 2, SBLK], f32, tag="sc",
                               name="qk_ps")[:, 0, :]
                for ko in range(KO):
                    nc.tensor.matmul(
                        pst,
                        w[m][:, ko, :],
                        hsT_q[si][:, ko, :],
                        start=(ko == 0), stop=(ko == KO - 1),
                    )
                if not with_bias:
                    if use_act:
                        nc.scalar.copy(dst, pst)
                    else:
                        nc.vector.tensor_copy(dst, pst)
                elif use_act:
                    nc.scalar.activation(dst, pst, Ident,
                                         bias=bcol[:, m:m + 1])
                else:
                    nc.vector.tensor_scalar_add(dst, pst, bcol[:, m:m + 1])

            def v_proj(to):
                pst = scp.tile([P, 2, SBLK], f32, tag="sc",
                               name="v_ps")[:, 0, 0:HG]
                for ko in range(KO):
                    nc.tensor.matmul(
                        pst,
                        hsT_q[to // 4][:, ko, (to % 4) * P:(to % 4 + 1) * P],
                        w_sb["v"][:, ko, :],
                        start=(ko == 0),
                        stop=(not with_bias and ko == KO - 1),
                    )
                if with_bias:
                    nc.tensor.matmul(
                        pst, ones_row[0:1, :], bv_row[:],
                        start=False, stop=True,
                    )
                nc.vector.tensor_copy(
                    v_t[:, to, :, 0:HD],
                    pst.rearrange("p (h d) -> p h d", d=HD),
                )

            # ---- attention pipeline ----------------------------------
            def _attention_pair(pair, unit_hook=None):
                kTt = qkT[f"k{pair}"]
                dve_set = DVE_SET if pair == 0 else DVE_SET_P1
                for sb_i in range(NB):
                    qTt = qT_s[(pair, sb_i)]
                    expt = ep.tile([P, NTCH, 2, SBLK], e_dt, tag="expt",
                                   name="expt")
                    ctxps = [cxp.tile([HD + 1, SBLK], f32, tag="cx",
                                      name=f"ctx{h2}") for h2 in range(2)]

                    def scores_exp(t):
                        sc = scp.tile([P, 2, SBLK], f32, tag="sc",
                                      name="sc")
                        for h2 in range(2):
                            po = 64 * h2
                            nc.tensor.matmul(
                                sc[:, h2, :],
                                kTt[po:po + HD, t * P:(t + 1) * P],
                                qTt[po:po + HD, :],
                                start=True, stop=True,
                                tile_position=(po, 0),
                            )
                        if t in dve_set:
                            if PV_FP8:
                                nc.vector.tensor_scalar(
                                    expt[:, t, :, :].bitcast(i8), sc[:],
                                    CM8, CB8, Mult, Add,
                                )
                            else:
                                nc.vector.tensor_scalar(
                                    expt[:, t, :, :].bitcast(i16), sc[:],
                                    CM16, CB16, Mult, Add,
                                )
                        else:
                            nc.scalar.activation(
                                expt[:, t, :, :], sc[:], Exp, scale=0.125,
                            )

                    def ctx_batch(ts):
                        # ts: even-aligned range; fp8 path consumes t-chunk
                        # PAIRS via DoubleRow (256-key contraction per MM)
                        for h2 in range(2):
                            head = pair * 2 + h2
                            if PV_FP8:
                                for tp in range(ts.start, ts.stop, 2):
                                    nc.tensor.matmul(
                                        ctxps[h2][:],
                                        v_t[:, tp:tp + 2, head, 0:HD + 1],
                                        expt[:, tp:tp + 2, h2, :],
                                        start=(tp == 0),
                                        stop=(tp == NTCH - 2),
                                        perf_mode=DR,
                                        skip_group_check=True,
                                    )
                            else:
                                for t in ts:
                                    nc.tensor.matmul(
                                        ctxps[h2][:],
                                        v_t[:, t, head, 0:HD + 1],
                                        expt[:, t, h2, :],
                                        start=(t == 0), stop=(t == NTCH - 1),
                                        skip_group_check=True,
                                    )

                    for t in range(NTCH):
                        scores_exp(t)
                        if unit_hook is not None:
                            unit_hook(sb_i, t)
                        if t in (5, 9, 13):
                            ctx_batch(range(t - 5, t - 1))
                        elif t == 15:
                            ctx_batch(range(12, 14))
                    ctx_batch(range(14, NTCH))

                    if TRANSPOSELESS:
                        # normalize in [d, s] layout: rec_row = 1/sumexp
                        # (fp16), broadcast across partitions via a K=1
                        # matmul, multiply, DMA out transposed.
                        for h2 in range(2):
                            head = pair * 2 + h2
                            ctxT = op.tile([HD + 1, SBLK], f32, tag="ctxT",
                                           name="ctxT")
                            nc.vector.tensor_copy(ctxT[:], ctxps[h2][:])
                            rec_row = op.tile([1, SBLK], f16, tag="rec",
                                              name="rec")
                            with nc.allow_low_precision(
                                    reason="fp16 reciprocal row; 2^-11 "
                                    "relative error is within budget"):
                                nc.vector.reciprocal(
                                    rec_row[:], ctxps[h2][HD:HD + 1, :])
                            recb = scp.tile([P, 2, SBLK], f32, tag="sc",
                                            name="recb")[0:HD, 0, :]
                            nc.tensor.matmul(
                                recb, ones_row[0:1, 0:HD], rec_row[:],
                                start=True, stop=True,
                            )
                            osbT = op.tile([HD, SBLK], f32, tag="osbT",
                                           name="osbT")
                            nc.vector.tensor_tensor(
                                osbT[:], ctxT[0:HD, :], recb,
                                mybir.AluOpType.mult,
                            )
                            nc.sync.dma_start(
                                out_d[head, :, sb_i * SBLK:(sb_i + 1) * SBLK],
                                osbT[:],
                            )
                    else:
                        ctxTs = []
                        for h2 in range(2):
                            ctxT = op.tile([HD + 1, SBLK], f32, tag="ctxT",
                                           name="ctxT")
                            if pair == 0:
                                nc.scalar.copy(ctxT[:], ctxps[h2][:])
                            else:
                                nc.vector.tensor_copy(ctxT[:], ctxps[h2][:])
                            ctxTs.append(ctxT)
                        for h2 in range(2):
                            head = pair * 2 + h2
                            ctxT = ctxTs[h2]
                            ot = cxp.tile([P, NB, HD + 1], f32, tag="cx",
                                          name="ot")
                            for c in range(NB):
                                nc.tensor.transpose(
                                    ot[:, c, :],
                                    ctxT[:, c * P:(c + 1) * P],
                                    ident[0:HD + 1, 0:HD + 1],
                                )
                            rec = op.tile([P, NB, 1], f32, tag="rec",
                                          name="rec")
                            nc.vector.reciprocal(rec[:], ot[:, :, HD:HD + 1])
                            osb = op.tile([P, NB, HD], f32, tag="osb",
                                          name="osb")
                            nc.vector.tensor_tensor(
                                osb[:], ot[:, :, 0:HD],
                                rec.to_broadcast([P, NB, HD]),
                                mybir.AluOpType.mult,
                            )
                            nc.sync.dma_start(
                                out_v[:, sb_i, :, head * HD:(head + 1) * HD],
                                osb[:],
                            )

            # ---- emission --------------------------------------------
            for si in range(NB):
                qk_proj(w_sb["k"], bcol_k, 0, si,
                        qkT["k0"][:, si * SBLK:(si + 1) * SBLK],
                        use_act=True)
            qk_proj(w_sb["q"], bcol_q, 0, 0, qT_s[(0, 0)][:],
                    use_act=True)

            def _hook_p0(sb_i, t):
                if sb_i == 0 and 1 <= t <= 8:
                    # two V chains per unit; group g is complete
                    # before ctx needs it two units later
                    v_proj(2 * (t - 1))
                    v_proj(2 * (t - 1) + 1)
                elif sb_i == 0 and 9 <= t <= 11:
                    qk_proj(w_sb["q"], bcol_q, 0, t - 8, qT_s[(0, t - 8)][:])
                elif sb_i == 1 and 1 <= t <= 4:
                    si = t - 1
                    qk_proj(w_sb["k"], bcol_k, 1, si,
                            qkT["k1"][:, si * SBLK:(si + 1) * SBLK])
                elif sb_i == 2 and 1 <= t <= 4:
                    qk_proj(w_sb["q"], bcol_q, 1, t - 1, qT_s[(1, t - 1)][:])

            _attention_pair(0, _hook_p0)
            _attention_pair(1)

            for _pool in (cxp, scp, op, ep):
                _pool.release()
    nc.compile()
    return nc


def _get_nc(with_bias=True):
    key = f"nc_{with_bias}"
    if key not in _CACHE:
        _CACHE[key] = _build_nc(with_bias=with_bias)
    return _CACHE[key]


def _kernel_np(hidden_states, attention_mask, Wq, bq, Wk, bk, Wv, bv):
    """Numpy fallback for the general (non-zero attention_mask) case."""
    S_, B_, H_ = hidden_states.shape
    hd = H_ // NH

    def split(x):
        return x.reshape(S_, B_ * NH, hd).transpose(1, 0, 2)

    q = split(hidden_states @ Wq + bq)
    k = split(hidden_states @ Wk + bk)
    v = split(hidden_states @ Wv + bv)
    scores = np.einsum("nsd,ntd->nst", q, k).reshape(B_, NH, S_, S_)
    scores = scores / np.sqrt(np.float32(hd)) + attention_mask
    scores = scores - scores.max(axis=-1, keepdims=True)
    e = np.exp(scores)
    probs = (e / e.sum(axis=-1, keepdims=True)).reshape(B_ * NH, S_, S_)
    ctx = np.einsum("nst,ntd->nsd", probs.astype(np.float32), v)
    return ctx.transpose(1, 0, 2).reshape(S_, B_, H_).astype(np.float32)


def kernel(hidden_states, attention_mask, Wq, bq, Wk, bk, Wv, bv, _trace=False, _tmpdir=None):
    import ml_dtypes
    bf = ml_dtypes.bfloat16
    f16 = np.float16
    f8 = ml_dtypes.float8_e4m3fn
    e_np = f8 if PV_FP8 else bf
    hidden_states = np.ascontiguousarray(hidden_states, dtype=np.float32)
    if attention_mask is not None and np.any(attention_mask):
        return _kernel_np(hidden_states, attention_mask, Wq, bq, Wk, bk, Wv, bv)

    from concourse.bass_utils import run_bass_kernel_spmd

    with_bias = bool(np.any(bq) or np.any(bk) or np.any(bv))
    nc = _get_nc(with_bias=with_bias)
    ones = np.ones(NTCH * NHEADS_CORE * P, e_np)
    onesb = np.ones(P, bf)
    idf = np.eye(P, dtype=np.float32)
    hs_16 = hidden_states.astype(bf)
    wq_16 = np.asarray(Wq, np.float32).astype(bf)
    wk_16 = np.asarray(Wk, np.float32).astype(bf)
    wv_16 = np.asarray(Wv, np.float32).astype(bf)

    def warr(w, c0):
        # [H, 256] -> [p, ko, m] contiguous
        return np.ascontiguousarray(
            w[:, c0:c0 + HG].reshape(KO, P, HG).transpose(1, 0, 2))

    in_maps = []
    for core in range(N_CORES):
        b = core // 4
        hg = core % 4
        c0 = hg * HG
        # hsT: [si, p, ko, s] with hs[si*512+s, ko*128+p]
        hsT = np.ascontiguousarray(
            hs_16[:, b, :].reshape(NB, SBLK, KO, P).transpose(0, 3, 2, 1))
        in_maps.append({
            "hsT": hsT,
            "wq": warr(wq_16, c0),
            "wk": warr(wk_16, c0),
            "wv": warr(wv_16, c0),
            "bq": np.ascontiguousarray(bq[c0:c0 + HG], dtype=np.float32),
            "bk": np.ascontiguousarray(bk[c0:c0 + HG], dtype=np.float32),
            "bv": np.ascontiguousarray(np.asarray(bv[c0:c0 + HG], np.float32).astype(bf)),
            "ones": ones,
            "onesb": onesb,
            "idf": idf,
        })
    res = None
    last_err = None
    for _attempt in range(3):
        try:
            res = run_bass_kernel_spmd(
                nc, in_maps, core_ids=list(range(N_CORES)), trace=_trace,
                tmpdir=_tmpdir,
            )
            break
        except Exception as e:  # transient NRT/device hiccups: retry
            last_err = e
            import time as _time
            _time.sleep(2.0)
    if res is None:
        raise last_err
    out = np.empty((S, B, H), np.float32)
    for core in range(N_CORES):
        b = core // 4
        hg = core % 4
        r = res.results[core]["out"]
        if TRANSPOSELESS:
            # r: [4, 64, S] -> [S, 256]
            out[:, b, hg * HG:(hg + 1) * HG] = r.reshape(HG, S).T
        else:
            out[:, b, hg * HG:(hg + 1) * HG] = r
    if _trace:
        _CACHE["last_results"] = res
    return out
